# revision 45
# baseline (speedup 1.0000x reference)
"""Last-query sparse attention on 8 TRN2 NeuronCores.

Reference computation (per sample b):
    prev  = x[b, :-1, :]                 # [T-1, D]
    final = x[b, -1, :]                  # [D]
    s     = prev @ final                 # [T-1]
    w     = softmax(s)
    att   = w @ prev                     # [D]
    out   = concat(final, att)           # [2D]

Sharding: batch (B=64) split 8 ways -> 8 samples per core, no collectives.

Final design (trace-driven; HW medians: this ~123us vs baseline ~127.5):
- DMA: 16-block (2MB) SWDGE cast chunks (8KB write packets run the SDMA
  engines at their ~26GB/s limit; 16KB packets measure ~20% slower).
  Samples 0-5 front-loaded (xbpool bufs=6); sample 0 ramps 4/4/8/16;
  sample 7 loads 16/8/4/4 so the last-arriving data needs minimal work.
- Pass 1 on DVE (the saturated engine, ~84us busy): chunk-split fp16
  mul+l1 (all 2x mode), merged l2/l3/l4 tree + 1x segmented reduce to
  fp16 S. gpsimd tensor ops measured 2.4x WORSE overall - never offload
  tree levels there.
- All 8 Fh broadcasts built up front (the in-order ACT FIFO otherwise
  parks them behind matmul-dependent stage copies).
- Softmax max for ALL samples via the deterministic PE-transpose chain
  (DVE row-max -> PE transpose -> DVE reduce_max on p0 -> PE
  ones-broadcast -> ACT negate -> ACT exp). gpsimd partition_all_reduce
  execution time varies 0.4-6.6us (Q7/SBUF contention with DVE 2x mode)
  and a late AR cascades into buffer releases and the DMA wire. The
  tail is NON-interleaved: in the in-order PE FIFO a chain emitted
  between two samples' matmul bursts would wait for the whole burst.
- Z epilogue split: ACT accum_out right behind each stage copy; the DVE
  reciprocal + ACT normalize deferred one sample (an early reciprocal
  waiting on a queued ACT accum blocks the whole in-order DVE stream).
- Pass 2: 32 PE matmuls/sample ([128,1] weights x [128,256] block) into
  a [1,512] PSUM row + ones-matmul Z column.
- Sample 7: 3 flash pieces (16/8/8; final piece's muls split 4+4 for
  data pipelining) with cumulative max, per-piece PSUM banks with
  zero-padded 16-wide Z, single end combine (2 STT adds over [0:272]),
  Z via one ACT accum_out.
"""

import sys

sys.path.insert(0, "/opt/trn_rl_repo")

from contextlib import ExitStack

import numpy as np

import concourse.tile as tile
import concourse.bass_isa as bass_isa
from concourse import bacc, mybir
from concourse.bass_utils import run_bass_kernel_spmd

N_CORES = 8
B = 64
T = 4096
D = 256
BPC = B // N_CORES  # samples per core
P = 128
NBLK = T // P  # 32 blocks; t = p*NBLK + i
CB = 16  # blocks per chunk
F32 = mybir.dt.float32
FP16 = mybir.dt.float16
AX = mybir.AxisListType
ALU = mybir.AluOpType

_NC_CACHE = None


def _build():
    AF = mybir.ActivationFunctionType
    nc = bacc.Bacc(
        trn_type="TRN2",
        target_bir_lowering=False,
        debug=False,
        num_devices=N_CORES,
    )
    x_ext = nc.declare_dram_parameter("x", [BPC, T, D], F32, isOutput=False)
    out_ext = nc.declare_dram_parameter("out", [BPC, 2 * D], F32, isOutput=True)
    xap = x_ext.ap()
    oap = out_ext.ap()

    with ExitStack() as ctx:
        tc = ctx.enter_context(tile.TileContext(nc))
        xbpool = ctx.enter_context(tc.tile_pool(name="xbp", bufs=6))
        fpool = ctx.enter_context(tc.tile_pool(name="fp", bufs=1))
        scrpool = ctx.enter_context(tc.tile_pool(name="scr", bufs=2))
        spool = ctx.enter_context(tc.tile_pool(name="sp", bufs=3))
        stat = ctx.enter_context(tc.tile_pool(name="stat", bufs=3))
        cpool = ctx.enter_context(tc.tile_pool(name="const", bufs=1))
        pspool = ctx.enter_context(tc.tile_pool(name="ps", bufs=4, space="PSUM"))
        psf_pool = ctx.enter_context(tc.tile_pool(name="psf", bufs=1, space="PSUM"))

        xr = [xap[b].rearrange("(p i) d -> p i d", p=P) for b in range(BPC)]

        # --- earliest DMAs ------------------------------------------------
        F_all = cpool.tile([1, BPC, D], F32, tag="fall")
        nc.sync.dma_start(F_all[0:1, :, :], xap[:, T - 1, :].unsqueeze(0))

        Xh = [xbpool.tile([P, NBLK, D], FP16, tag="xh", name="xh") for _ in range(BPC)]

        def trig(b, lo, hi):
            nc.gpsimd.dma_start(Xh[b][:, lo:hi, :], xr[b][:, lo:hi, :])

        trig(0, 0, 4)
        trig(0, 4, 8)
        trig(0, 8, 16)
        trig(0, 16, 32)
        for b in range(1, 6):
            trig(b, 0, CB)
            trig(b, CB, NBLK)

        # final half of the output: straight HBM->HBM copy (sync engine)
        nc.sync.dma_start(oap[:, 0:D], xap[:, T - 1, :])

        # --- constants ----------------------------------------------------
        ones16 = cpool.tile([P, 1], FP16, tag="ones16")
        nc.vector.memset(ones16[:], 1.0)
        onesf = cpool.tile([1, P], F32, tag="onesf")
        nc.vector.memset(onesf[:], 1.0)
        maskbias = cpool.tile([P, 1], FP16, tag="mb")
        nc.vector.memset(maskbias[:], 0.0)
        neg6e4 = cpool.tile([1, 1], FP16, tag="neg6e4")
        nc.vector.memset(neg6e4[:], -60000.0)
        nc.sync.dma_start(maskbias[P - 1 : P, 0:1], neg6e4[0:1, 0:1])
        # identity for PE-transpose (tail max chain)
        ident = cpool.tile([P, P], F32, tag="ident")
        nc.gpsimd.memset(ident[:], 0.0)
        nc.gpsimd.affine_select(
            out=ident[:],
            in_=ident[:],
            compare_op=ALU.not_equal,
            fill=1.0,
            base=0,
            pattern=[[-1, P]],
            channel_multiplier=1,
        )

        stage = cpool.tile([1, BPC, 288], F32, tag="stage")
        att_n = cpool.tile([1, BPC, D], F32, tag="attn")

        # all 8 Fh broadcasts up front: PE ones-matmul + ACT cast copy
        Fh = []
        for b in range(BPC):
            ps = psf_pool.tile([P, D], F32, tag="psf", name=f"psfb{b}")
            nc.tensor.matmul(ps[:], lhsT=onesf[:], rhs=F_all[0:1, b, :], start=True, stop=True)
            fh = fpool.tile([P, D], FP16, tag=f"fh{b}", name=f"fh{b}")
            nc.scalar.activation(fh[:], ps[:], AF.Copy)
            Fh.append(fh)

        def keep_warm(anchor_ap, idx):
            pass

        def pe_max_chain(rm, tagsfx):
            # deterministic cross-partition max: rm [P,1] -> PE transpose ->
            # DVE reduce_max on partition 0 -> scalar [1,1]
            pmax = psf_pool.tile([1, P], F32, tag="pmax", name=f"pmax{tagsfx}")
            nc.tensor.transpose(pmax[:], rm[:], ident[:])
            gls = stat.tile([1, 1], F32, tag=f"gl{tagsfx}", name=f"gl{tagsfx}")
            nc.vector.reduce_max(gls[:], pmax[0:1, :], axis=AX.X)
            return gls

        def bcast_neg(gsc, tagsfx):
            # scalar [1,1] -> [P,1] negated bias for the ACT exp
            pbc = psf_pool.tile([P, 1], F32, tag="pbc", name=f"pbc{tagsfx}")
            nc.tensor.matmul(pbc[:], lhsT=onesf[:], rhs=gsc[:], start=True, stop=True)
            negk = stat.tile([P, 1], F32, tag=f"ng{tagsfx}", name=f"ng{tagsfx}")
            nc.scalar.activation(negk[:], pbc[:], AF.Copy, scale=-1.0)
            return negk

        rz = stat.tile([1, BPC], F32, tag="rz", bufs=1)
        zbs = [None] * BPC

        def z_accum(b):
            # Z(b) via ACT accum_out, emitted right behind the stage copy
            zb = stat.tile([1, 1], F32, tag=f"zb{b}", name=f"zb{b}")
            zscr = stat.tile([1, NBLK], F32, tag="zscr", name="zscr")
            nc.scalar.activation(
                zscr[0:1, 0:NBLK], stage[0:1, b, D : D + NBLK], AF.Copy,
                accum_out=zb[:],
            )
            zbs[b] = zb

        def finish(b):
            # reciprocal + normalize, emitted only once Z(b) is surely done
            nc.vector.reciprocal(rz[0:1, b : b + 1], zbs[b][:])
            nc.scalar.activation(
                att_n[0:1, b, :], stage[0:1, b, 0:D], AF.Copy,
                scale=rz[0:1, b : b + 1],
            )

        def mul_l1(b, lo, hi, prod, l1):
            w = hi - lo
            nc.vector.tensor_mul(
                prod[:, lo:hi, :],
                Xh[b][:, lo:hi, :],
                Fh[b][:].unsqueeze(1).broadcast_to((P, w, D)),
            )
            nc.vector.tensor_add(
                l1[:, lo:hi, :], prod[:, lo:hi, 0 : D // 2], prod[:, lo:hi, D // 2 : D]
            )

        def tree_tail(lo, hi, l1, l2, l3, l4, S):
            nc.vector.tensor_add(
                l2[:, lo:hi, :], l1[:, lo:hi, 0 : D // 4], l1[:, lo:hi, D // 4 : D // 2]
            )
            nc.vector.tensor_add(
                l3[:, lo:hi, :], l2[:, lo:hi, 0 : D // 8], l2[:, lo:hi, D // 8 : D // 4]
            )
            nc.vector.tensor_add(
                l4[:, lo:hi, :], l3[:, lo:hi, 0 : D // 16], l3[:, lo:hi, D // 16 : D // 8]
            )
            with nc.allow_low_precision(reason="scores tree is already fp16"):
                nc.vector.reduce_sum(S[:, lo:hi], l4[:, lo:hi, :], axis=AX.X)

        def mask_self(S):
            nc.vector.tensor_add(
                S[:, NBLK - 1 : NBLK], S[:, NBLK - 1 : NBLK], maskbias[:]
            )

        def pass2(Pw, bank, b, lo, hi, zw):
            for i in range(lo, hi):
                nc.tensor.matmul(
                    bank[0:1, 0:D],
                    lhsT=Pw[:, i - lo : i - lo + 1],
                    rhs=Xh[b][:, i, :],
                    start=(i == lo),
                    stop=(i == hi - 1),
                )
            nc.tensor.matmul(
                bank[0:1, D : D + zw], lhsT=ones16[:], rhs=Pw[:, 0:zw],
                start=True, stop=True,
            )

        def scr_tiles():
            prod = scrpool.tile([P, NBLK, D], FP16, tag="prod", name="prod")
            l1 = scrpool.tile([P, NBLK, D // 2], FP16, tag="l1", name="l1")
            l2 = scrpool.tile([P, NBLK, D // 4], FP16, tag="l2", name="l2")
            l3 = scrpool.tile([P, NBLK, D // 8], FP16, tag="l3", name="l3")
            l4 = scrpool.tile([P, NBLK, D // 16], FP16, tag="l4", name="l4")
            return prod, l1, l2, l3, l4

        # ------------------- samples 0..5 (steady state) -------------------
        for b in range(6):
            S = spool.tile([P, NBLK], FP16, tag="s")
            prod, l1, l2, l3, l4 = scr_tiles()

            if b == 0:
                mul_l1(b, 0, 4, prod, l1)
                mul_l1(b, 4, 8, prod, l1)
                mul_l1(b, 8, 16, prod, l1)
                mul_l1(b, 16, 32, prod, l1)
            else:
                mul_l1(b, 0, CB, prod, l1)
                mul_l1(b, CB, NBLK, prod, l1)
            tree_tail(0, NBLK, l1, l2, l3, l4, S)

            mask_self(S)
            if b == 5:
                keep_warm(S[:, 0:4], "5w")
            rm = stat.tile([P, 1], F32, tag="rm")
            nc.vector.reduce_max(rm[:], S[:], axis=AX.X)
            # PE-chain max everywhere: partition_all_reduce execution time
            # varies 0.4-6.6us (Q7/SBUF contention with DVE 2x mode) and a
            # late AR cascades into buffer releases and the DMA wire
            gb = pe_max_chain(rm, f"{b}")
            negmax = bcast_neg(gb, f"{b}")
            if b == 1:
                trig(6, 0, CB)
                trig(6, CB, NBLK)
            elif b == 2:
                trig(7, 0, CB)
                trig(7, CB, 24)
                trig(7, 24, 28)
                trig(7, 28, 32)

            Pw = spool.tile([P, NBLK], FP16, tag="pw")
            nc.scalar.activation(Pw[:], S[:], AF.Exp, bias=negmax[:], scale=1.0)

            bank = pspool.tile([1, 512], F32, tag="attps", name="attps")
            pass2(Pw, bank, b, 0, NBLK, NBLK)
            nc.scalar.activation(stage[0:1, b, 0:288], bank[0:1, 0:288], AF.Copy)
            z_accum(b)
            if b > 0:
                finish(b - 1)

        # ------------------- sample 6: PE-chain softmax --------------------
        S6 = spool.tile([P, NBLK], FP16, tag="s", name="s6")
        prod6, l16, l26, l36, l46 = scr_tiles()
        mul_l1(6, 0, CB, prod6, l16)
        mul_l1(6, CB, NBLK, prod6, l16)
        tree_tail(0, NBLK, l16, l26, l36, l46, S6)
        mask_self(S6)
        keep_warm(S6[:, 0:4], "6w")
        rm6 = stat.tile([P, 1], F32, tag="rm", name="rm6")
        nc.vector.reduce_max(rm6[:], S6[:], axis=AX.X)
        g6 = pe_max_chain(rm6, "6")
        ng6 = bcast_neg(g6, "6")
        Pw6 = spool.tile([P, NBLK], FP16, tag="pw", name="pw6")
        nc.scalar.activation(Pw6[:], S6[:], AF.Exp, bias=ng6[:], scale=1.0)
        bank6 = pspool.tile([1, 512], F32, tag="attps", name="attps6")
        pass2(Pw6, bank6, 6, 0, NBLK, NBLK)
        nc.scalar.activation(stage[0:1, 6, 0:288], bank6[0:1, 0:288], AF.Copy)
        z_accum(6)
        finish(5)

        # ------------------- sample 7: 3-piece flash tail ------------------
        S7 = spool.tile([P, NBLK], FP16, tag="s", name="s7")
        prod7, l17, l27, l37, l47 = scr_tiles()
        PIECES = [(0, 16), (16, 24), (24, 32)]
        gs = []
        banks = []
        sbs = []
        gcs = None
        for k, (lo, hi) in enumerate(PIECES):
            w = hi - lo
            if k == 2:
                mul_l1(7, 24, 28, prod7, l17)
                mul_l1(7, 28, 32, prod7, l17)
            else:
                mul_l1(7, lo, hi, prod7, l17)
            tree_tail(lo, hi, l17, l27, l37, l47, S7)
            if hi == NBLK:
                mask_self(S7)
            keep_warm(S7[:, lo : lo + 4], f"7p{k}")
            rmk = stat.tile([P, 1], F32, tag=f"rm7{k}", name=f"rm7{k}")
            nc.vector.reduce_max(rmk[:], S7[:, lo:hi], axis=AX.X)
            gls = pe_max_chain(rmk, f"7{k}")
            if k > 0:
                gnew = stat.tile([1, 1], F32, tag=f"gc7{k}", name=f"gc7{k}")
                nc.vector.tensor_max(gnew[:], gls[:], gcs[:])
                gcs = gnew
            else:
                gcs = gls
            gs.append(gcs)
            negk = bcast_neg(gcs, f"7{k}")
            Pwk = spool.tile([P, CB], FP16, tag=f"pw7{k}", name=f"pw7{k}")
            if w < CB:
                nc.vector.memset(Pwk[:, w:CB], 0.0)
            nc.scalar.activation(
                Pwk[:, 0:w], S7[:, lo:hi], AF.Exp, bias=negk[:], scale=1.0
            )
            bk = pspool.tile([1, 512], F32, tag="attps", name=f"att7{k}")
            pass2(Pwk[:, 0:CB], bk, 7, lo, hi, CB)
            banks.append(bk)
            if k < len(PIECES) - 1:
                sbk = stat.tile([1, D + CB], F32, tag=f"sb7{k}", name=f"sb7{k}")
                nc.scalar.activation(sbk[:], bk[0:1, 0 : D + CB], AF.Copy)
                sbs.append(sbk)
            if k == 1:
                # sample 6's finish here, not at k==0: its Z waits on the
                # matmul-gated stage copy, and a DVE reciprocal popping
                # before that lands blocks the final pieces' muls
                finish(6)
        # alphas a_k = exp(gm_k - gmF); final piece used the true max
        negF = stat.tile([1, 1], F32, tag="negF", name="negF")
        nc.scalar.activation(negF[:], gcs[:], AF.Copy, scale=-1.0)
        alphas = []
        for k in range(2):
            ak = stat.tile([1, 1], F32, tag=f"al7{k}", name=f"al7{k}")
            nc.scalar.activation(
                ak[:], gs[k][:], AF.Exp, bias=negF[0:1, 0:1], scale=1.0
            )
            alphas.append(ak)
        # combine att+Z in one strip: stage7 = a0*sb0 + a1*sb1 + bank2
        u1 = stat.tile([1, D + CB], F32, tag="u1", name="u1")
        nc.vector.scalar_tensor_tensor(
            out=u1[:], in0=sbs[1][:], scalar=alphas[1][0:1, 0:1],
            in1=banks[2][0:1, 0 : D + CB], op0=ALU.mult, op1=ALU.add,
        )
        nc.vector.scalar_tensor_tensor(
            out=stage[0:1, 7, 0 : D + CB], in0=sbs[0][:],
            scalar=alphas[0][0:1, 0:1],
            in1=u1[:], op0=ALU.mult, op1=ALU.add,
        )
        zb7 = stat.tile([1, 1], F32, tag="zb7", name="zb7")
        zscr7 = stat.tile([1, CB], F32, tag="z7scr", name="z7scr")
        nc.scalar.activation(
            zscr7[:], stage[0:1, 7, D : D + CB], AF.Copy, accum_out=zb7[:]
        )
        nc.vector.reciprocal(rz[0:1, 7:8], zb7[:])
        nc.scalar.activation(
            att_n[0:1, 7, :], stage[0:1, 7, 0:D], AF.Copy,
            scale=rz[0:1, 7:8],
        )

        nc.sync.dma_start(
            oap[0 : BPC - 1, D : 2 * D].unsqueeze(0), att_n[0:1, 0 : BPC - 1, :]
        )
        nc.sync.dma_start(
            oap[BPC - 1 : BPC, D : 2 * D].unsqueeze(0), att_n[0:1, BPC - 1, :].unsqueeze(1)
        )

    nc.compile()
    return nc


def _run(x, trace=False):
    global _NC_CACHE
    x = np.ascontiguousarray(np.asarray(x, dtype=np.float32))
    assert x.shape == (B, T, D), x.shape
    if _NC_CACHE is None:
        _NC_CACHE = _build()
    in_maps = [{"x": x[c * BPC : (c + 1) * BPC]} for c in range(N_CORES)]
    res = run_bass_kernel_spmd(
        _NC_CACHE, in_maps, core_ids=list(range(N_CORES)), trace=trace
    )
    out = np.concatenate([res.results[c]["out"] for c in range(N_CORES)], axis=0)
    return out.astype(np.float32), res


def kernel(x):
    out, _ = _run(x, trace=False)
    return out
